# revision 1
# baseline (speedup 1.0000x reference)
"""NeuralODE (nn_NeuralODE_36807869727439) Trainium2 Bass kernel, 8 NeuronCores.

Math: 26 Euler steps of z += h * (tanh(z@W1 + b1 + t*u) @ W2 + b2), with
B=256, D=2048, H=4096 and the step grid derived from the input t exactly as
the reference does.

Distribution scheme (tensor-parallel over H, one AllGather per step):
  * Track p = z @ W1 (+ accumulated bias drift) instead of z.  With
    G = W2 @ W1 (host-precomputed, fp16) the recurrence is
        a_k = tanh(p_k + c_k),   p_{k+1} = p_k + h_k * (a_k @ G)
    where c_k = b1 + t_k*u + (sum_{j<k} h_j) * (b2@W1) is host-precomputed.
    Since H = 2D, a_k @ G has exactly the same FLOPs as the two original
    GEMMs per step.
  * Core i holds column shard G[:, 512i:512(i+1)] (fp16, 4MB, SBUF-resident)
    and the state shard p[:, H_i] in T-layout [512, 256] fp32.
  * Per step, each core computes ha = h*tanh(p+c) (fp16, 128KB), and one
    8-rank AllGather assembles ha_full [4096, B] for the GEMM rhs.
    The batch is split in two halves -> two independent software pipelines,
    so each half's GEMM/tanh hides under the other half's AllGather.
  * z_final = z0 + (sum_k h_k a_k) @ W2 + (sum h_k) b2 is linear in the a_k:
    each core accumulates S = sum h_k a_k for its H-shard (fp32, DVE), runs
    one fp32 GEMM against W2[H_i, :] at the end, and the host sums the eight
    [2048, 256] partials (no z exchange, no final collective).
"""
import math
import sys

import numpy as np

if "/opt/trn_rl_repo" not in sys.path:
    sys.path.insert(0, "/opt/trn_rl_repo")

B = 256
D = 2048
H = 4096
N_CORES = 8
H_LOC = H // N_CORES          # 512
H_MAX = 0.05                  # ODEsolver_Euler default max step
KCH = H // 128                # 32 contraction chunks
MT = H_LOC // 128             # 4 m-tiles per core


def _compute_schedule(t):
    """Mirror reference._euler_solve stepping exactly (fp64 interval math,
    fp32 h and fp32 accumulated t)."""
    t64 = np.asarray(t, dtype=np.float64)
    sched = []
    for i in range(t64.shape[0] - 1):
        t0, t1 = t64[i], t64[i + 1]
        n = int(math.ceil(abs(t1 - t0) / H_MAX))
        if n == 0:
            continue
        h = np.float32((t1 - t0) / n)
        tc = np.float32(t0)
        for _ in range(n):
            tc = np.float32(tc + h)
            sched.append((float(h), float(tc)))
    return sched


def _host_prepare(z0, W1, b1, u, W2, b2, sched):
    f32, f16, f64 = np.float32, np.float16, np.float64
    nsteps = len(sched)
    G16 = (W2.astype(f64) @ W1.astype(f64)).astype(f16)          # [H, H]
    b2W1 = (b2.astype(f64) @ W1.astype(f64)).astype(f32)         # [H]
    p0 = z0.astype(f32) @ W1.astype(f32)                         # [B, H]
    hs = np.array([h for h, _ in sched], dtype=f32)
    cumh = np.concatenate([[0.0], np.cumsum(hs.astype(f64))[:-1]]).astype(f32)
    ts = np.array([tc for _, tc in sched], dtype=f32)
    cbias = (b1[None, :].astype(f32)
             + ts[:, None] * u[None, :].astype(f32)
             + cumh[:, None] * b2W1[None, :])                    # [nsteps, H]
    # step-0 gathered payload host-computed: the first step needs no AllGather
    ha0 = (hs[0] * np.tanh(p0 + cbias[0])).astype(f16)           # [B, H]
    ha0_dev = np.ascontiguousarray(
        ha0.T.reshape(KCH, 128, B).transpose(1, 0, 2))           # [128, KCH, B]

    in_maps = []
    for i in range(N_CORES):
        hlo = H_LOC * i
        Gc = G16[:, hlo:hlo + H_LOC]
        Gc_dev = np.ascontiguousarray(
            Gc.reshape(KCH, 128, H_LOC).transpose(1, 0, 2))      # [128, 32, 512]
        p0T = p0[:, hlo:hlo + H_LOC].T
        p0_dev = np.ascontiguousarray(p0T.reshape(MT, 128, B))   # [4, 128, 256]
        cb = cbias[:, hlo:hlo + H_LOC]
        cb_dev = np.ascontiguousarray(
            cb.reshape(nsteps, MT, 128).transpose(2, 0, 1).reshape(128, nsteps * MT))
        W2r = W2[hlo:hlo + H_LOC, :].astype(f32)
        W2r_dev = np.ascontiguousarray(W2r.reshape(MT, 128, D))  # [4, 128, 2048]
        in_maps.append({
            "g_in": Gc_dev,
            "p0_in": p0_dev,
            "cb_in": cb_dev,
            "w2_in": W2r_dev,
            "ha0_in": ha0_dev,
        })
    return in_maps


def _build_program(sched, split=2, haf_group=16):
    import concourse.bacc as bacc
    import concourse.mybir as mybir
    import concourse.tile as tile

    nsteps = len(sched)
    nc = bacc.Bacc("TRN2", target_bir_lowering=False, debug=False,
                   num_devices=N_CORES)

    g_in = nc.dram_tensor("g_in", [128, KCH, H_LOC], mybir.dt.float16, kind="ExternalInput")
    p0_in = nc.dram_tensor("p0_in", [MT, 128, B], mybir.dt.float32, kind="ExternalInput")
    cb_in = nc.dram_tensor("cb_in", [128, nsteps * MT], mybir.dt.float32, kind="ExternalInput")
    w2_in = nc.dram_tensor("w2_in", [MT, 128, D], mybir.dt.float32r, kind="ExternalInput")
    ha0_in = nc.dram_tensor("ha0_in", [128, KCH, B], mybir.dt.float16, kind="ExternalInput")
    zf_out = nc.dram_tensor("zf_out", [D // 128, 128, B], mybir.dt.float32, kind="ExternalOutput")

    BS = B // split
    with tile.TileContext(nc) as tc:
        with (
            tc.tile_pool(name="sbuf", bufs=1) as pool,
            tc.tile_pool(name="psum", bufs=1, space="PSUM") as psum_pool,
            tc.tile_pool(name="dram", bufs=1, space="DRAM") as dram_pool,
        ):
            G_sb = pool.tile([128, KCH, H_LOC], mybir.dt.float16, tag="G_sb")
            nc.scalar.dma_start(G_sb[:], g_in[:])
            cb_sb = pool.tile([128, nsteps * MT], mybir.dt.float32, tag="cb_sb")
            nc.sync.dma_start(cb_sb[:], cb_in[:])
            p_sb = pool.tile([128, MT, B], mybir.dt.float32, tag="p_sb")
            for m in range(MT):
                nc.sync.dma_start(p_sb[:, m, :], p0_in[m])
            S_sb = pool.tile([128, MT, B], mybir.dt.float32, tag="S_sb")
            nc.vector.memset(S_sb[:], 0.0)

            def produce_ha(k, hx, m, ha_sb, ag_i):
                cs = hx * BS
                h_k = sched[k][0]
                a_t = pool.tile([128, BS], mybir.dt.float32,
                                tag=f"a_t{hx}{m}", bufs=2, name=f"a_{k}_{hx}_{m}")
                nc.scalar.activation(
                    a_t[:], p_sb[:, m, cs:cs + BS],
                    mybir.ActivationFunctionType.Tanh,
                    bias=cb_sb[:, k * MT + m:k * MT + m + 1],
                )
                nc.vector.tensor_scalar_mul(ha_sb[:, m * BS:(m + 1) * BS], a_t[:],
                                            float(h_k))
                nc.vector.tensor_tensor(
                    S_sb[:, m, cs:cs + BS], S_sb[:, m, cs:cs + BS],
                    ha_sb[:, m * BS:(m + 1) * BS], mybir.AluOpType.add,
                )
                if ag_i is not None:
                    nc.sync.dma_start(
                        ag_i[m * 128:(m + 1) * 128, :],
                        ha_sb[:, m * BS:(m + 1) * BS])

            def new_ha_buffers(k, hx, with_agi=True):
                ha_sb = pool.tile([128, MT * BS], mybir.dt.float16,
                                  tag=f"ha_sb{hx}", bufs=2, name=f"ha_{k}_{hx}")
                ag_i = None
                if with_agi:
                    ag_i = dram_pool.tile([H_LOC, BS], mybir.dt.float16,
                                          tag=f"agi_{k}_{hx}", name=f"agi_{k}_{hx}")
                return ha_sb, ag_i

            anchors = []
            haf0 = pool.tile([128, KCH, B], mybir.dt.float16, tag="hafz")
            nc.scalar.dma_start(haf0[:], ha0_in[:])
            staged = {}
            for hx in range(split):
                ha_sb, _ = new_ha_buffers(0, hx, with_agi=False)
                for m in range(MT):
                    produce_ha(0, hx, m, ha_sb, None)

            # the last step's AG+GEMM would only produce p_n, never read
            for k in range(nsteps - 1):
                for hx in range(split):
                    cs = hx * BS
                    if k == 0:
                        haf = haf0[:, :, cs:cs + BS]
                    else:
                        ag_i = staged[hx]
                        ag_o = dram_pool.tile([H, BS], mybir.dt.float16,
                                              tag=f"ago_{k}_{hx}", name=f"ago_{k}_{hx}",
                                              addr_space="Shared")
                        nc.gpsimd.collective_compute(
                            "AllGather", mybir.AluOpType.bypass,
                            replica_groups=[list(range(N_CORES))],
                            ins=[ag_i[:].opt()],
                            outs=[ag_o[:].opt()],
                        )
                        haf_t = pool.tile([128, KCH, BS], mybir.dt.float16,
                                          tag=f"haf{hx}", bufs=3, name=f"haf_{k}_{hx}")
                        dma_engines = [nc.scalar, nc.scalar]
                        for g in range(KCH // haf_group):
                            dma_engines[g % len(dma_engines)].dma_start(
                                haf_t[:, g * haf_group:(g + 1) * haf_group, :],
                                ag_o[g * haf_group * 128:(g + 1) * haf_group * 128, :]
                                   .rearrange("(c p) b -> p c b", p=128),
                            )
                        haf = haf_t[:]
                    ps = psum_pool.tile([128, MT * BS], mybir.dt.float32,
                                        tag=f"ps{hx}", bufs=2, name=f"ps_{k}_{hx}")
                    need_agi = k + 1 <= nsteps - 2
                    ha_next, agi_next = new_ha_buffers(k + 1, hx, with_agi=need_agi)
                    for m in range(MT):
                        for kk in range(KCH):
                            nc.tensor.matmul(
                                ps[:, m * BS:(m + 1) * BS],
                                G_sb[:, kk, m * 128:(m + 1) * 128],
                                haf[:, kk, :],
                                start=(kk == 0), stop=(kk == KCH - 1),
                            )
                        pupd = nc.vector.tensor_tensor(
                            p_sb[:, m, cs:cs + BS], p_sb[:, m, cs:cs + BS],
                            ps[:, m * BS:(m + 1) * BS], mybir.AluOpType.add,
                        )
                        if k == nsteps * 3 // 4 and hx == 0 and m == 0:
                            anchors.append(pupd.ins)
                        produce_ha(k + 1, hx, m, ha_next, agi_next)
                    staged[hx] = agi_next

            from concourse.tile import add_dep_helper
            w2_sb = pool.tile([128, MT, D], mybir.dt.float32r, tag="w2_sb")
            for m in range(MT):
                w2dma = nc.gpsimd.dma_start(w2_sb[:, m, :], w2_in[m])
                if anchors:
                    add_dep_helper(anchors[0], w2dma.ins, sync=False,
                                   reason="load w2 late")
            S_r = pool.tile([128, MT, B], mybir.dt.float32r, tag="S_r")
            nc.vector.tensor_copy(S_r[:], S_sb[:])
            for mt in range(D // 128):
                psf = psum_pool.tile([128, B], mybir.dt.float32,
                                     tag=f"psf{mt % 4}", bufs=1, name=f"psf_{mt}")
                for kk in range(MT):
                    nc.tensor.matmul(
                        psf[:],
                        w2_sb[:, kk, mt * 128:(mt + 1) * 128],
                        S_r[:, kk, :],
                        start=(kk == 0), stop=(kk == MT - 1),
                    )
                zf_sb = pool.tile([128, B], mybir.dt.float32,
                                  tag=f"zf_sb{mt % 4}", bufs=1, name=f"zf_sb_{mt}")
                nc.vector.tensor_copy(zf_sb[:], psf[:])
                nc.sync.dma_start(zf_out[mt], zf_sb[:])

    nc.compile()
    return nc


_PROGRAM_CACHE = {}


def kernel(z0, t, W1, b1, u, W2, b2):
    from concourse.bass_utils import run_bass_kernel_spmd

    z0 = np.asarray(z0)
    t = np.asarray(t)
    W1 = np.asarray(W1)
    b1 = np.asarray(b1)
    u = np.asarray(u)
    W2 = np.asarray(W2)
    b2 = np.asarray(b2)

    sched = _compute_schedule(t)
    if not sched:
        return z0.astype(np.float32).copy()

    key = tuple(sched)
    nc = _PROGRAM_CACHE.get(key)
    if nc is None:
        nc = _build_program(sched)
        _PROGRAM_CACHE[key] = nc
    in_maps = _host_prepare(z0, W1, b1, u, W2, b2, sched)
    res = run_bass_kernel_spmd(nc, in_maps, list(range(N_CORES)))

    f32 = np.float32
    acc = np.zeros((D, B), dtype=f32)
    for r in res.results:
        acc += r["zf_out"].reshape(D, B)
    sumh = f32(np.sum(np.array([h for h, _ in sched], dtype=f32), dtype=np.float64))
    out = z0.astype(f32) + acc.T + sumh * b2.astype(f32)
    return out.astype(np.float32)



# revision 3
# speedup vs baseline: 1.2136x; 1.2136x over previous
"""NeuralODE (nn_NeuralODE_36807869727439) Trainium2 Bass kernel, 8 NeuronCores.

Math: n Euler steps (n=26 for the given t grid) of
    z += h_k * (tanh(z@W1 + b1 + t_k*u) @ W2 + b2),
B=256, D=2048, H=4096; schedule derived from t exactly as the reference.

Scheme (tensor-parallel over H, one fp8 AllGather per step per batch half):
  * Track q = s*(z@W1 + c_k) with s = 2^16, where c_k = b1 + t_k*u +
    cumh_k*(b2@W1) is the per-step affine drift.  With G = W2@W1:
        a_k = tanh(q_k / s),   q_{k+1} = q_k + x_k @ Gq + s*(c_{k+1}-c_k)
    where x_k = e4m3(32*h_k*a_k) and Gq = e4m3(G*s/32), so all per-step h
    variation (h in {0.05, 0.0333}) rides in the sent activations.
  * Core i holds q[:, H_i] (H_i = 512 cols) in batch-major layout
    [128 batch x 512] fp32, LIVING IN PSUM: the step GEMMs accumulate into
    it directly (start=False), no vector-engine state update at all.
  * The GEMM orientation makes the gathered activations the STATIONARY
    operand (transposed x, fp8 DoubleRow [128,2,128] chunks) and G the
    MOVING operand ([128,2,512] fp8): full N=512 streaming, 2 MACs/cycle.
  * x is transposed batch-major -> H-major by the DMA XBAR transpose
    (dma_start_transpose, off the PE), scaled to fp8 by the vector engine,
    and AllGathered (64 KB/rank).  The two batch halves run as independent
    software-pipelined chains so each AllGather hides under the other
    half's GEMM.
  * S = sum_k x_k accumulates on the vector engine; the final
    zf = S @ (W2/32) GEMM runs in bf16; host adds z0 + sum_h*b2 and the 8
    D-sharded partials.
"""
import math
import sys

import numpy as np
import ml_dtypes

if "/opt/trn_rl_repo" not in sys.path:
    sys.path.insert(0, "/opt/trn_rl_repo")

B = 256
D = 2048
H = 4096
N_CORES = 8
H_LOC = H // N_CORES          # 512
H_MAX = 0.05                  # ODEsolver_Euler default max step
KCH2 = H // 256               # 16 double-row contraction chunks
S_E = 65536.0                 # 2^16 state scale
SA = 32.0                     # activation send scale

E4 = ml_dtypes.float8_e4m3    # == TRN fp8_e4m3 (max +-240)
BF16 = ml_dtypes.bfloat16


def _compute_schedule(t):
    """Mirror reference._euler_solve stepping exactly (fp64 interval math,
    fp32 h and fp32 accumulated t)."""
    t64 = np.asarray(t, dtype=np.float64)
    sched = []
    for i in range(t64.shape[0] - 1):
        t0, t1 = t64[i], t64[i + 1]
        n = int(math.ceil(abs(t1 - t0) / H_MAX))
        if n == 0:
            continue
        h = np.float32((t1 - t0) / n)
        tc = np.float32(t0)
        for _ in range(n):
            tc = np.float32(tc + h)
            sched.append((float(h), float(tc)))
    return sched


def _host_prepare(z0, W1, b1, u, W2, b2, sched):
    f32, f16, f64 = np.float32, np.float16, np.float64
    n = len(sched)
    G64 = W2.astype(f64) @ W1.astype(f64)                       # [H, H]
    b2W1 = (b2.astype(f64) @ W1.astype(f64)).astype(f32)        # [H]
    hs = np.array([h for h, _ in sched], dtype=f32)
    ts = np.array([tc for _, tc in sched], dtype=f32)
    cumh = np.concatenate([[0.0], np.cumsum(hs.astype(f64))[:-1]]).astype(f32)
    c = (b1[None, :].astype(f32)
         + ts[:, None] * u[None, :].astype(f32)
         + cumh[:, None] * b2W1[None, :])                       # [n, H]
    c0 = c[0] * f32(S_E)
    dc = (c[1:] - c[:-1]) * f32(S_E) if n > 1 else np.zeros((1, H), f32)

    Gq = np.clip(G64 * (S_E / SA), -240.0, 240.0).astype(E4)    # [H, H] fp8
    z0t = np.ascontiguousarray(
        z0.T.reshape(D // 128, 128, B).transpose(1, 0, 2)).astype(f16)

    in_maps = []
    for i in range(N_CORES):
        hlo = H_LOC * i
        g = Gq[:, hlo:hlo + H_LOC]                              # [H, 512]
        g_dev = np.ascontiguousarray(
            g.reshape(KCH2, 2, 128, H_LOC).transpose(2, 0, 1, 3))  # [128,16,2,512]
        w1_dev = np.ascontiguousarray(
            (W1[:, hlo:hlo + H_LOC].astype(f32) * f32(S_E))
            .reshape(D // 128, 128, H_LOC).transpose(1, 0, 2)).astype(f16)
        c0_dev = c0[hlo:hlo + H_LOC].astype(f16)[None, :]       # [1, 512]
        dc_dev = np.ascontiguousarray(
            dc[:, hlo:hlo + H_LOC].astype(f16))[None]           # [1, n-1, 512]
        w2_dev = np.ascontiguousarray(
            (W2[hlo:hlo + H_LOC, :].astype(f32) * f32(1.0 / SA))
            .reshape(4, 128, D).transpose(1, 0, 2)).astype(BF16)  # [128,4,2048]
        in_maps.append({
            "g_in": g_dev,
            "z0t_in": z0t,
            "w1_in": w1_dev,
            "c0_in": c0_dev,
            "dc_in": dc_dev,
            "w2_in": w2_dev,
        })
    return in_maps


def _build_program(sched):
    import concourse.bacc as bacc
    import concourse.mybir as mybir
    import concourse.tile as tile

    n = len(sched)
    n_dc = max(n - 1, 1)
    nc = bacc.Bacc("TRN2", target_bir_lowering=False, debug=False,
                   num_devices=N_CORES)

    g_in = nc.dram_tensor("g_in", [128, KCH2, 2, H_LOC], mybir.dt.float8e4, kind="ExternalInput")
    z0t_in = nc.dram_tensor("z0t_in", [128, D // 128, B], mybir.dt.float16, kind="ExternalInput")
    w1_in = nc.dram_tensor("w1_in", [128, D // 128, H_LOC], mybir.dt.float16, kind="ExternalInput")
    c0_in = nc.dram_tensor("c0_in", [1, H_LOC], mybir.dt.float16, kind="ExternalInput")
    dc_in = nc.dram_tensor("dc_in", [1, n_dc, H_LOC], mybir.dt.float16, kind="ExternalInput")
    w2_in = nc.dram_tensor("w2_in", [128, 4, D], mybir.dt.bfloat16, kind="ExternalInput")
    zf_out = nc.dram_tensor("zf_out", [D // 128, 128, 2, 128], mybir.dt.float32, kind="ExternalOutput")

    DR = mybir.MatmulPerfMode.DoubleRow

    with tile.TileContext(nc) as tc:
        with (
            tc.tile_pool(name="sbuf", bufs=1) as pool,
            tc.tile_pool(name="psum", bufs=1, space="PSUM") as psum_pool,
            tc.tile_pool(name="dram", bufs=1, space="DRAM") as dram_pool,
        ):
            G_sb = pool.tile([128, KCH2, 2, H_LOC], mybir.dt.float8e4, tag="G_sb")
            nc.scalar.dma_start(G_sb[:], g_in[:])
            z0t_sb = pool.tile([128, D // 128, B], mybir.dt.float16, tag="z0t_sb")
            nc.sync.dma_start(z0t_sb[:], z0t_in[:])
            w1_sb = pool.tile([128, D // 128, H_LOC], mybir.dt.float16, tag="w1_sb")
            nc.sync.dma_start(w1_sb[:], w1_in[:])
            c0_sb = pool.tile([1, H_LOC], mybir.dt.float16, tag="c0_sb")
            nc.sync.dma_start(c0_sb[:], c0_in[:])
            dc_sb = pool.tile([1, n_dc, H_LOC], mybir.dt.float16, tag="dc_sb")
            nc.sync.dma_start(dc_sb[:], dc_in[:])
            w2_sb = pool.tile([128, 4, D], mybir.dt.bfloat16, tag="w2_sb")
            nc.gpsimd.dma_start(w2_sb[:], w2_in[:])
            ones_sb = pool.tile([1, 128], mybir.dt.float16, tag="ones_sb")
            nc.vector.memset(ones_sb[:], 1.0)
            S_sb = pool.tile([128, 2, 4, 128], mybir.dt.float32, tag="S_sb")
            nc.vector.memset(S_sb[:], 0.0)

            Q = [psum_pool.tile([128, H_LOC], mybir.dt.float32, tag=f"Q{h}",
                                name=f"Q_{h}")
                 for h in range(2)]

            # q0 = s*(z0@W1 + c0) straight into PSUM (start=True opens the bank)
            for h in range(2):
                for kk in range(D // 128):
                    nc.tensor.matmul(
                        Q[h][:], z0t_sb[:, kk, 128 * h:128 * (h + 1)],
                        w1_sb[:, kk, :],
                        start=(kk == 0), stop=False, skip_group_check=True)
                nc.tensor.matmul(Q[h][:], ones_sb[:, :], c0_sb[:, :],
                                 start=False, stop=True, skip_group_check=True)

            def produce(k, h):
                """tanh -> xbar-transpose -> fp8 scale -> S accum; for
                non-final steps also AllGather; returns gathered af tile."""
                h_k = sched[k][0]
                a = pool.tile([128, H_LOC], mybir.dt.bfloat16,
                              tag=f"a{h}", bufs=2, name=f"a_{k}_{h}")
                nc.scalar.activation(a[:], Q[h][:],
                                     mybir.ActivationFunctionType.Tanh,
                                     scale=float(1.0 / S_E))
                at = pool.tile([128, 4, 128], mybir.dt.bfloat16,
                               tag=f"at{h}", bufs=2, name=f"at_{k}_{h}")
                nc.sync.dma_start_transpose(at[:], a[:])
                x = pool.tile([128, 4, 128], mybir.dt.float8e4,
                              tag=f"x{h}", bufs=2, name=f"x_{k}_{h}")
                nc.vector.tensor_scalar_mul(x[:], at[:], float(SA * h_k))
                nc.vector.tensor_tensor(S_sb[:, h], S_sb[:, h], x[:],
                                        mybir.AluOpType.add)
                if k >= n - 1:
                    return None
                ag_i = dram_pool.tile([H_LOC, 128], mybir.dt.float8e4,
                                      tag=f"agi_{k}_{h}", name=f"agi_{k}_{h}")
                nc.sync.dma_start(
                    ag_i[:].rearrange("(j p) b -> p j b", p=128), x[:])
                ag_o = dram_pool.tile([H, 128], mybir.dt.float8e4,
                                      tag=f"ago_{k}_{h}", name=f"ago_{k}_{h}",
                                      addr_space="Shared")
                nc.gpsimd.collective_compute(
                    "AllGather", mybir.AluOpType.bypass,
                    replica_groups=[list(range(N_CORES))],
                    ins=[ag_i[:].opt()],
                    outs=[ag_o[:].opt()],
                )
                af = pool.tile([128, KCH2, 2, 128], mybir.dt.float8e4,
                               tag=f"af{h}", bufs=2, name=f"af_{k}_{h}")
                src = ag_o[:].rearrange("(kk i2 p) b -> p kk i2 b", p=128, i2=2)
                half = KCH2 // 2
                nc.scalar.dma_start(af[:, :half], src[:, :half])
                nc.scalar.dma_start(af[:, half:], src[:, half:])
                return af

            def gemm(k, h, af):
                """q_{k+1} accumulate: drift + x_k @ Gq (DoubleRow fp8)."""
                nc.tensor.matmul(Q[h][:], ones_sb[:, :], dc_sb[:, k, :],
                                 start=False, stop=False, skip_group_check=True)
                for kk in range(KCH2):
                    nc.tensor.matmul(
                        Q[h][:], af[:, kk], G_sb[:, kk],
                        start=False, stop=(kk == KCH2 - 1),
                        perf_mode=DR, skip_group_check=True)

            if n == 1:
                produce(0, 0)
                produce(0, 1)
            else:
                af_a = produce(0, 0)
                af_b = None
                for k in range(n - 1):
                    if k > 0:
                        gemm(k - 1, 1, af_b)
                    af_b = produce(k, 1)
                    gemm(k, 0, af_a)
                    af_a = produce(k + 1, 0)
                gemm(n - 2, 1, af_b)
                produce(n - 1, 1)

            # zf = S @ (W2/32), bf16
            Sb_sb = pool.tile([128, 2, 4, 128], mybir.dt.bfloat16, tag="Sb_sb")
            nc.vector.tensor_copy(Sb_sb[:], S_sb[:])
            for mt in range(D // 128):
                psf = psum_pool.tile([128, 2, 128], mybir.dt.float32,
                                     tag=f"psf{mt % 2}", bufs=1, name=f"psf_{mt}")
                for kk in range(4):
                    nc.tensor.matmul(
                        psf[:],
                        w2_sb[:, kk, 128 * mt:128 * (mt + 1)],
                        Sb_sb[:, :, kk, :],
                        start=(kk == 0), stop=(kk == 3))
                zf_sb = pool.tile([128, 2, 128], mybir.dt.float32,
                                  tag=f"zf{mt % 2}", bufs=2, name=f"zf_{mt}")
                nc.vector.tensor_copy(zf_sb[:], psf[:])
                nc.sync.dma_start(zf_out[mt], zf_sb[:])

    nc.compile()
    return nc


_PROGRAM_CACHE = {}


def kernel(z0, t, W1, b1, u, W2, b2):
    from concourse.bass_utils import run_bass_kernel_spmd

    z0 = np.asarray(z0)
    t = np.asarray(t)
    W1 = np.asarray(W1)
    b1 = np.asarray(b1)
    u = np.asarray(u)
    W2 = np.asarray(W2)
    b2 = np.asarray(b2)

    sched = _compute_schedule(t)
    if not sched:
        return z0.astype(np.float32).copy()

    key = tuple(sched)
    nc = _PROGRAM_CACHE.get(key)
    if nc is None:
        nc = _build_program(sched)
        _PROGRAM_CACHE[key] = nc
    in_maps = _host_prepare(z0, W1, b1, u, W2, b2, sched)
    res = run_bass_kernel_spmd(nc, in_maps, list(range(N_CORES)))

    f32 = np.float32
    acc = np.zeros((D // 128, 128, 2, 128), dtype=f32)
    for r in res.results:
        acc += r["zf_out"].astype(f32)
    # acc[mt, p, hh, b] = dz[b + 128*hh, 128*mt + p]
    dz = acc.transpose(2, 3, 0, 1).reshape(B, D)
    sumh = f32(np.sum(np.array([h for h, _ in sched], dtype=f32), dtype=np.float64))
    out = z0.astype(f32) + dz + sumh * b2.astype(f32)
    return out.astype(np.float32)


# revision 5
# speedup vs baseline: 1.6414x; 1.3525x over previous
"""NeuralODE (nn_NeuralODE_36807869727439) Trainium2 Bass kernel, 8 NeuronCores.

Math: n Euler steps (n=26 for the given t grid) of
    z += h_k * (tanh(z@W1 + b1 + t_k*u) @ W2 + b2),
B=256, D=2048, H=4096; schedule derived from t exactly as the reference.

Scheme (tensor-parallel over H, one fp8 AllGather per step per batch half):
  * Track q = s*(z@W1 + c_k) with s = 2^15, where c_k = b1 + t_k*u +
    cumh_k*(b2@W1).  With G = W2@W1 and the step sizes h grouped into a
    few distinct values (0.05 / 0.0333...), precompute per-group
    Gq[v] = e4m3(G*s*h_v).  Per step:
        a_k = e4m3(tanh(q_k / s)),  q_{k+1} = q_k + a_k @ Gq[v(k)] + s*dc_k
    Core i holds q[:, H_i] (H_i = 512 cols) batch-major [128 x 512] fp32
    LIVING IN PSUM -- the GEMMs accumulate into it (start=False), no
    vector-engine state update.
  * GEMM orientation: gathered fp8 activations are the STATIONARY operand
    (DoubleRow [128,2,128] chunks), Gq the MOVING operand ([128,2,512]):
    N=512 streaming at 2 fp8 MACs/cycle.  The drift s*dc_k enters as a
    K=1 fp16 matmul.
  * Per step/half: tanh -> fp8 (scalar engine), 4 PE transposes -> PSUM,
    copy to SBUF, DMA to DRAM, mesh AllGather (64 KB/rank), gathered
    load split in 2 chunks so the GEMM starts on the first.  The two
    batch halves are independent chains on dedicated DMA queues
    (half A: scalar, half B: sync) so each AllGather hides under the
    other half's GEMM.
  * S_v = sum_{k in group v} a_k accumulates on the vector engine;
    final zf = (sum_v h_v S_v) @ W2 runs in bf16; host adds z0 + sumh*b2
    and the 8 D-sharded partials.
"""
import math
import sys

import numpy as np
import ml_dtypes

if "/opt/trn_rl_repo" not in sys.path:
    sys.path.insert(0, "/opt/trn_rl_repo")

B = 256
D = 2048
H = 4096
N_CORES = 8
H_LOC = H // N_CORES          # 512
H_MAX = 0.05                  # ODEsolver_Euler default max step
KCH2 = H // 256               # 16 double-row contraction chunks
S_E = 32768.0                 # 2^15 state scale

E4 = ml_dtypes.float8_e4m3    # == TRN fp8_e4m3 (max +-240)
BF16 = ml_dtypes.bfloat16


def _compute_schedule(t):
    """Mirror reference._euler_solve stepping exactly (fp64 interval math,
    fp32 h and fp32 accumulated t)."""
    t64 = np.asarray(t, dtype=np.float64)
    sched = []
    for i in range(t64.shape[0] - 1):
        t0, t1 = t64[i], t64[i + 1]
        n = int(math.ceil(abs(t1 - t0) / H_MAX))
        if n == 0:
            continue
        h = np.float32((t1 - t0) / n)
        tc = np.float32(t0)
        for _ in range(n):
            tc = np.float32(tc + h)
            sched.append((float(h), float(tc)))
    return sched


def _h_groups(sched):
    """Cluster the step sizes h (fp32-exact values differ in the last ulp)
    into groups; returns (group mean h list, per-step group index)."""
    uniq = []
    idx = []
    for h, _ in sched:
        gi = None
        for j, hv in enumerate(uniq):
            if abs(h - hv[0]) <= 1e-4 * abs(hv[0]):
                gi = j
                break
        if gi is None:
            uniq.append([h])
            gi = len(uniq) - 1
            idx.append(gi)
        else:
            uniq[gi].append(h)
            idx.append(gi)
    means = [float(np.mean(np.array(g, dtype=np.float64))) for g in uniq]
    return means, idx


def _host_prepare(z0, W1, b1, u, W2, b2, sched):
    f32, f16, f64 = np.float32, np.float16, np.float64
    n = len(sched)
    hmeans, _ = _h_groups(sched)
    G64 = W2.astype(f64) @ W1.astype(f64)                       # [H, H]
    b2W1 = (b2.astype(f64) @ W1.astype(f64)).astype(f32)        # [H]
    hs = np.array([h for h, _ in sched], dtype=f32)
    ts = np.array([tc for _, tc in sched], dtype=f32)
    cumh = np.concatenate([[0.0], np.cumsum(hs.astype(f64))[:-1]]).astype(f32)
    c = (b1[None, :].astype(f32)
         + ts[:, None] * u[None, :].astype(f32)
         + cumh[:, None] * b2W1[None, :])                       # [n, H]
    c0 = c[0] * f32(S_E)
    dc = (c[1:] - c[:-1]) * f32(S_E) if n > 1 else np.zeros((1, H), f32)

    Gq = [np.clip(G64 * (S_E * hv), -240.0, 240.0).astype(E4) for hv in hmeans]
    z0t = np.ascontiguousarray(
        z0.T.reshape(D // 128, 128, B).transpose(1, 0, 2)).astype(f16)
    ident = np.eye(128, dtype=np.float32).astype(BF16)

    in_maps = []
    for i in range(N_CORES):
        hlo = H_LOC * i
        m = {
            "z0t_in": z0t,
            "ident_in": ident,
            "c0_in": c0[hlo:hlo + H_LOC].astype(f16)[None, :],
            "dc_in": np.ascontiguousarray(
                dc[:, hlo:hlo + H_LOC].astype(f16))[None],
            "w1_in": np.ascontiguousarray(
                (W1[:, hlo:hlo + H_LOC].astype(f32) * f32(S_E))
                .reshape(D // 128, 128, H_LOC).transpose(1, 0, 2)).astype(f16),
            "w2_in": np.ascontiguousarray(
                W2[hlo:hlo + H_LOC, :].astype(f32)
                .reshape(4, 128, D).transpose(1, 0, 2)).astype(BF16),
        }
        for v, g in enumerate(Gq):
            gc = g[:, hlo:hlo + H_LOC]                          # [H, 512]
            m[f"g{v}_in"] = np.ascontiguousarray(
                gc.reshape(KCH2, 2, 128, H_LOC).transpose(2, 0, 1, 3))
        in_maps.append(m)
    return in_maps


def _build_program(sched):
    import concourse.bacc as bacc
    import concourse.mybir as mybir
    import concourse.tile as tile

    n = len(sched)
    n_dc = max(n - 1, 1)
    hmeans, hidx = _h_groups(sched)
    nv = len(hmeans)
    nc = bacc.Bacc("TRN2", target_bir_lowering=False, debug=False,
                   num_devices=N_CORES)

    g_ins = [nc.dram_tensor(f"g{v}_in", [128, KCH2, 2, H_LOC],
                            mybir.dt.float8e4, kind="ExternalInput")
             for v in range(nv)]
    z0t_in = nc.dram_tensor("z0t_in", [128, D // 128, B], mybir.dt.float16, kind="ExternalInput")
    ident_in = nc.dram_tensor("ident_in", [128, 128], mybir.dt.bfloat16, kind="ExternalInput")
    w1_in = nc.dram_tensor("w1_in", [128, D // 128, H_LOC], mybir.dt.float16, kind="ExternalInput")
    c0_in = nc.dram_tensor("c0_in", [1, H_LOC], mybir.dt.float16, kind="ExternalInput")
    dc_in = nc.dram_tensor("dc_in", [1, n_dc, H_LOC], mybir.dt.float16, kind="ExternalInput")
    w2_in = nc.dram_tensor("w2_in", [128, 4, D], mybir.dt.bfloat16, kind="ExternalInput")
    zf_out = nc.dram_tensor("zf_out", [D // 128, 128, 2, 128], mybir.dt.float32, kind="ExternalOutput")

    DR = mybir.MatmulPerfMode.DoubleRow

    with tile.TileContext(nc) as tc:
        with (
            tc.tile_pool(name="sbuf", bufs=1) as pool,
            tc.tile_pool(name="psum", bufs=1, space="PSUM") as psum_pool,
            tc.tile_pool(name="dram", bufs=1, space="DRAM") as dram_pool,
        ):
            G_sb = []
            for v in range(nv):
                g_t = pool.tile([128, KCH2, 2, H_LOC], mybir.dt.float8e4,
                                tag=f"G{v}_sb", name=f"G{v}_sb")
                nc.scalar.dma_start(g_t[:], g_ins[v][:])
                G_sb.append(g_t)
            z0t_sb = pool.tile([128, D // 128, B], mybir.dt.float16, tag="z0t_sb")
            nc.sync.dma_start(z0t_sb[:], z0t_in[:])
            ident_sb = pool.tile([128, 128], mybir.dt.bfloat16, tag="ident_sb")
            nc.sync.dma_start(ident_sb[:], ident_in[:])
            w1_sb = pool.tile([128, D // 128, H_LOC], mybir.dt.float16, tag="w1_sb")
            nc.sync.dma_start(w1_sb[:], w1_in[:])
            c0_sb = pool.tile([1, H_LOC], mybir.dt.float16, tag="c0_sb")
            nc.sync.dma_start(c0_sb[:], c0_in[:])
            dc_sb = pool.tile([1, n_dc, H_LOC], mybir.dt.float16, tag="dc_sb")
            nc.sync.dma_start(dc_sb[:], dc_in[:])
            w2_sb = pool.tile([128, 4, D], mybir.dt.bfloat16, tag="w2_sb")
            nc.gpsimd.dma_start(w2_sb[:], w2_in[:])
            ones_sb = pool.tile([1, 128], mybir.dt.float16, tag="ones_sb")
            nc.vector.memset(ones_sb[:], 1.0)
            S_sb = pool.tile([128, nv, 2, 4, 128], mybir.dt.float32, tag="S_sb")
            nc.vector.memset(S_sb[:], 0.0)

            Q = [psum_pool.tile([128, H_LOC], mybir.dt.float32, tag=f"Q{h}",
                                name=f"Q_{h}")
                 for h in range(2)]
            TP = [psum_pool.tile([128, 4, 128], mybir.dt.bfloat16, tag=f"TP{h}",
                                 name=f"TP_{h}")
                  for h in range(2)]
            dmae = [nc.scalar, nc.sync]   # per-half DMA queues

            # q0 = s*(z0@W1 + c0) straight into PSUM (start=True opens bank)
            for h in range(2):
                for kk in range(D // 128):
                    nc.tensor.matmul(
                        Q[h][:], z0t_sb[:, kk, 128 * h:128 * (h + 1)],
                        w1_sb[:, kk, :],
                        start=(kk == 0), stop=False, skip_group_check=True)
                nc.tensor.matmul(Q[h][:], ones_sb[:, :], c0_sb[:, :],
                                 start=False, stop=True, skip_group_check=True)

            def produce(k, h):
                """tanh->fp8, PE transpose, stage, AllGather; returns af."""
                v = hidx[k]
                a8 = pool.tile([128, H_LOC], mybir.dt.bfloat16,
                               tag=f"a{h}", bufs=2, name=f"a_{k}_{h}")
                nc.scalar.activation(a8[:], Q[h][:],
                                     mybir.ActivationFunctionType.Tanh,
                                     scale=float(1.0 / S_E))
                for j in range(4):
                    nc.tensor.transpose(TP[h][:, j, :],
                                        a8[:, 128 * j:128 * (j + 1)],
                                        ident_sb[:])
                x = pool.tile([128, 4, 128], mybir.dt.float8e4,
                              tag=f"x{h}", bufs=2, name=f"x_{k}_{h}")
                nc.vector.tensor_copy(x[:], TP[h][:])
                nc.vector.tensor_tensor(S_sb[:, v, h], S_sb[:, v, h], x[:],
                                        mybir.AluOpType.add)
                if k >= n - 1:
                    return None
                ag_i = dram_pool.tile([128, H_LOC], mybir.dt.float8e4,
                                      tag=f"agi_{k}_{h}", name=f"agi_{k}_{h}")
                dmae[h].dma_start(ag_i[:], x[:])
                ag_o = dram_pool.tile([N_CORES * 128, H_LOC], mybir.dt.float8e4,
                                      tag=f"ago_{k}_{h}", name=f"ago_{k}_{h}",
                                      addr_space="Shared")
                nc.gpsimd.collective_compute(
                    "AllGather", mybir.AluOpType.bypass,
                    replica_groups=[list(range(N_CORES))],
                    ins=[ag_i[:].opt()],
                    outs=[ag_o[:].opt()],
                )
                af = pool.tile([128, N_CORES, 4, 128], mybir.dt.float8e4,
                               tag=f"af{h}", bufs=2, name=f"af_{k}_{h}")
                src = ag_o[:].rearrange("(c p) (j b) -> p c j b", p=128, b=128)
                half = N_CORES // 2
                dmae[h].dma_start(af[:, :half], src[:, :half])
                dmae[h].dma_start(af[:, half:], src[:, half:])
                return af

            def gemm(k, h, af):
                """q_{k+1} accumulate: drift + a_k @ Gq (DoubleRow fp8)."""
                v = hidx[k]
                nc.tensor.matmul(Q[h][:], ones_sb[:, :], dc_sb[:, k, :],
                                 start=False, stop=False, skip_group_check=True)
                for kk in range(KCH2):
                    j0 = 2 * (kk % 2)
                    nc.tensor.matmul(
                        Q[h][:], af[:, kk // 2, j0:j0 + 2, :], G_sb[v][:, kk],
                        start=False, stop=(kk == KCH2 - 1),
                        perf_mode=DR, skip_group_check=True)

            if n == 1:
                produce(0, 0)
                produce(0, 1)
            else:
                af_a = produce(0, 0)
                af_b = None
                for k in range(n - 1):
                    if k > 0:
                        gemm(k - 1, 1, af_b)
                    af_b = produce(k, 1)
                    gemm(k, 0, af_a)
                    af_a = produce(k + 1, 0)
                gemm(n - 2, 1, af_b)
                produce(n - 1, 1)

            # Sw = sum_v h_v * S_v ; zf = Sw @ W2 in bf16
            Sw_sb = pool.tile([128, 2, 4, 128], mybir.dt.float32, tag="Sw_sb")
            nc.vector.tensor_scalar_mul(Sw_sb[:], S_sb[:, 0], float(hmeans[0]))
            for v in range(1, nv):
                Sv_sb = pool.tile([128, 2, 4, 128], mybir.dt.float32,
                                  tag="Sv_sb", name=f"Sv_{v}")
                nc.vector.tensor_scalar_mul(Sv_sb[:], S_sb[:, v], float(hmeans[v]))
                nc.vector.tensor_tensor(Sw_sb[:], Sw_sb[:], Sv_sb[:],
                                        mybir.AluOpType.add)
            Sb_sb = pool.tile([128, 2, 4, 128], mybir.dt.bfloat16, tag="Sb_sb")
            nc.vector.tensor_copy(Sb_sb[:], Sw_sb[:])
            for mt in range(D // 128):
                psf = psum_pool.tile([128, 2, 128], mybir.dt.float32,
                                     tag=f"psf{mt % 2}", bufs=1, name=f"psf_{mt}")
                for kk in range(4):
                    nc.tensor.matmul(
                        psf[:],
                        w2_sb[:, kk, 128 * mt:128 * (mt + 1)],
                        Sb_sb[:, :, kk, :],
                        start=(kk == 0), stop=(kk == 3))
                zf_sb = pool.tile([128, 2, 128], mybir.dt.float32,
                                  tag=f"zf{mt % 2}", bufs=2, name=f"zf_{mt}")
                nc.vector.tensor_copy(zf_sb[:], psf[:])
                nc.sync.dma_start(zf_out[mt], zf_sb[:])

    nc.compile()
    return nc


_PROGRAM_CACHE = {}


def kernel(z0, t, W1, b1, u, W2, b2):
    from concourse.bass_utils import run_bass_kernel_spmd

    z0 = np.asarray(z0)
    t = np.asarray(t)
    W1 = np.asarray(W1)
    b1 = np.asarray(b1)
    u = np.asarray(u)
    W2 = np.asarray(W2)
    b2 = np.asarray(b2)

    sched = _compute_schedule(t)
    if not sched:
        return z0.astype(np.float32).copy()

    key = tuple(sched)
    nc = _PROGRAM_CACHE.get(key)
    if nc is None:
        nc = _build_program(sched)
        _PROGRAM_CACHE[key] = nc
    in_maps = _host_prepare(z0, W1, b1, u, W2, b2, sched)
    res = run_bass_kernel_spmd(nc, in_maps, list(range(N_CORES)))

    f32 = np.float32
    acc = np.zeros((D // 128, 128, 2, 128), dtype=f32)
    for r in res.results:
        acc += r["zf_out"].astype(f32)
    # acc[mt, p, hh, b] = dz[b + 128*hh, 128*mt + p]
    dz = acc.transpose(2, 3, 0, 1).reshape(B, D)
    sumh = f32(np.sum(np.array([h for h, _ in sched], dtype=f32), dtype=np.float64))
    out = z0.astype(f32) + dz + sumh * b2.astype(f32)
    return out.astype(np.float32)
